# revision 25
# baseline (speedup 1.0000x reference)
"""Trainium2 Bass kernel for ComplexAttention.

Math (per (b,t) pair):
    cur2 = [cur_r, cur_i]                       # [2D]
    Q    = cur2 @ qW + qb                       # [D]
    K_s  = H_s @ kW + kb ; V_s = H_s @ vW + vb  # H = [hist_r, hist_i]  [S, 2D]
    sc_s = (Q . K_s) * scale * conf
    w    = softmax(sc) ; ctx = sum_s w_s V_s
    out  = cur + 0.1 * ctx (complex)

Rewrites used (exact):
    Q . K_s = (cur2 @ (qW kW^T) + qb kW^T) . H_s  +  (Q . kb)
        - the (Q . kb) term is constant over s -> softmax-invariant -> dropped.
    conf * scale is folded into Qk on the host (cur2t columns and the bias
        row), so the kernel computes sc directly.
    ctx = inv * (sum_s e_s H_s) @ vW + vb,  inv = 1/sum_s e_s
        - the hbar weighted sum is kept UNNORMALIZED on device; 1/den is
          applied at the final output stage (per-partition scalar), and
          0.1*vb is added on the host after gathering.

Structure per 128-pair batch (4 per core), per 32-pair sub-batch (4):
    - H tile [128, 8, E]: partitions = (pair-in-group j, slot s), 8 groups.
      hist_r DMA'd on the SP queue, hist_i on the ACT queue.
    - qkr (per group): PE replicates Qk rows across the 32 slots via the
      r32 selection constant (lhsT base = rhs base = 32*sb, so qk_t is
      consumed in place - no relayout).
    - scores: tensor_tensor(prod) + tensor_scalar(accum) per group,
      alternating DVE / Pool engines.
    - hbar: per-group [128, 32] zero-padded weight matmuls accumulate all
      8 groups into ONE [32, E] PSUM tile; a single ACT copy converts to
      f16 and a single DMA places it into hbar_b.  (The baseline's
      per-group [4, E] activation + gather DMA was the HW bottleneck.)
    - den: dnT = exp8^T @ m4 -> [8, 4]; ACT copy (x10) -> DMA scatter into
      dn128 [128, 1]; reciprocal -> 0.1/den; applied to ctx at the end.
"""

import os
import sys

import numpy as np

os.environ.setdefault("MYCRO_LOCAL_CACHE", "1")

try:
    import concourse.bass as bass
except ImportError:  # pragma: no cover
    sys.path.insert(0, "/opt/trn_rl_repo")
    import concourse.bass as bass

import concourse.mybir as mybir
import concourse.tile as tile
from concourse import bacc
from concourse.bass_utils import run_bass_kernel_spmd

F32 = mybir.dt.float32
F32R = mybir.dt.float32r
F16 = mybir.dt.float16
AX = mybir.AluOpType
AF = mybir.ActivationFunctionType

B, T, S, D = 4, 1024, 32, 512
D2 = 2 * D  # 1024, concat(real, imag) feature dim
E = 2 * D   # 1024, history feature dim
N_CORES = 8
PAIRS = B * T
SCALE = float(D) ** -0.5


BENCH_LOOP = int(os.environ.get("K_BENCH_LOOP", "0"))  # >0: repeat body N times
# timing ablations (invalid numerics, valid timing):
#   "dmaonly" - only the H-stream DMAs + token out write
#   "honce"   - load H once per batch (sb==0), reuse for sb 1-3
#   "noscore" - skip qkr replication matmuls + score DVE/Pool ops
ABLATE = os.environ.get("K_ABLATE", "")
DEBUG_STOP = os.environ.get("K_DEBUG_STOP", "")  # "", "A", "B", "DN", "HB"


def build_dmaonly(ppc: int) -> bass.Bass:
    """Timing ablation: only the H-stream DMAs + a token out write."""
    nb = ppc // 128
    nc = bacc.Bacc("TRN2", target_bir_lowering=False)
    hist_r = nc.declare_dram_parameter("hist_real", [ppc, S, D], F16, isOutput=False)
    hist_i = nc.declare_dram_parameter("hist_imag", [ppc, S, D], F16, isOutput=False)
    out = nc.declare_dram_parameter("out", [ppc, D, 2], F32, isOutput=True)

    from contextlib import nullcontext

    with tile.TileContext(nc) as tc:
        with tc.tile_pool(name="h", bufs=3) as hpool, \
             tc.tile_pool(name="scr", bufs=1) as spool:
            loop_cm = tc.For_i(0, BENCH_LOOP, 1) if BENCH_LOOP > 0 else nullcontext()
            with loop_cm:
                scr = spool.tile([128, 8], F32R)
                last = None
                for b in range(nb):
                    for sb in range(4):
                        h_t = hpool.tile([128, 8, E], F16, tag="h")
                        p0 = 128 * b + 32 * sb
                        nc.sync.dma_start(
                            out=h_t[:, :, 0:D],
                            in_=hist_r[p0 : p0 + 32]
                            .rearrange("(gl j) s d -> (j s) gl d", j=4),
                        )
                        nc.scalar.dma_start(
                            out=h_t[:, :, D:E],
                            in_=hist_i[p0 : p0 + 32]
                            .rearrange("(gl j) s d -> (j s) gl d", j=4),
                        )
                        last = h_t
                # token consumer keeps the last DMA ordered before kernel end
                nc.vector.tensor_copy(out=scr[:], in_=last[:, 0, 0:8])
                nc.sync.dma_start(
                    out=out[:]
                    .rearrange("(bb p) d two -> p bb (d two)", p=128)[:, 0, 0:8]
                    .bitcast(F32R),
                    in_=scr[:],
                )
    nc.compile()
    return nc


def build(ppc: int) -> bass.Bass:
    """Build the per-core SPMD program for `ppc` pairs per core."""
    if ABLATE == "dmaonly":
        return build_dmaonly(ppc)
    assert ppc % 128 == 0
    nb = ppc // 128      # batches of 128 pairs

    nc = bacc.Bacc("TRN2", target_bir_lowering=False)

    hist_r = nc.declare_dram_parameter("hist_real", [ppc, S, D], F16, isOutput=False)
    hist_i = nc.declare_dram_parameter("hist_imag", [ppc, S, D], F16, isOutput=False)
    cur_r = nc.declare_dram_parameter("cur_r", [ppc, D], F32, isOutput=False)
    cur_i = nc.declare_dram_parameter("cur_i", [ppc, D], F32, isOutput=False)
    cur2t = nc.declare_dram_parameter("cur2t", [D2, ppc], F32, isOutput=False)
    confr = nc.declare_dram_parameter("confr", [1, ppc], F32, isOutput=False)
    wqk = nc.declare_dram_parameter("wqk", [D2, E], F32, isOutput=False)
    bqk = nc.declare_dram_parameter("bqk", [1, E], F32, isOutput=False)
    vw = nc.declare_dram_parameter("vw", [E, E], F16, isOutput=False)
    sel = nc.declare_dram_parameter("sel", [128, 32, 128], F16, isOutput=False)
    m4 = nc.declare_dram_parameter("m4", [128, 4], F32, isOutput=False)
    ident = nc.declare_dram_parameter("ident", [128, 128], F16, isOutput=False)
    wd0 = nc.declare_dram_parameter("wd0", [128, 8, 32], F16, isOutput=False)
    ones1 = nc.declare_dram_parameter("ones1", [128, 8], F16, isOutput=False)
    out = nc.declare_dram_parameter("out", [ppc, D, 2], F32, isOutput=True)

    from contextlib import ExitStack, nullcontext

    with tile.TileContext(nc) as tc, ExitStack() as es:
            ec = es.enter_context
            cpool = ec(tc.tile_pool(name="const", bufs=1))
            wpool = ec(tc.tile_pool(name="bigw", bufs=1))
            c2pool = ec(tc.tile_pool(name="c2t", bufs=1))
            hpool = ec(tc.tile_pool(name="h", bufs=3))
            qkpool = ec(tc.tile_pool(name="qk", bufs=4))
            curpool = ec(tc.tile_pool(name="cur", bufs=2))
            prodpool = ec(tc.tile_pool(name="prod", bufs=2))
            qkspool = ec(tc.tile_pool(name="qkrs", bufs=2))
            smpool = ec(tc.tile_pool(name="sm", bufs=4))
            dnpool = ec(tc.tile_pool(name="dn", bufs=2))
            hsbpool = ec(tc.tile_pool(name="hsb", bufs=2))
            hbpool = ec(tc.tile_pool(name="hbarb", bufs=2))
            htpool = ec(tc.tile_pool(name="hbarT", bufs=8))
            outpool = ec(tc.tile_pool(name="outp", bufs=2))
            ps_rep = ec(tc.tile_pool(name="ps_rep", bufs=2, space="PSUM"))
            ps_hb = ec(tc.tile_pool(name="ps_hb", bufs=1, space="PSUM"))
            ps_sh = ec(tc.tile_pool(name="ps_sh", bufs=2, space="PSUM"))
            del es
            loop_cm = (
                tc.For_i(0, BENCH_LOOP, 1) if BENCH_LOOP > 0 else nullcontext()
            )
            with loop_cm:
                # ---- constants / weights resident in SBUF ----
                m4_t = cpool.tile([128, 4], F32)
                nc.sync.dma_start(out=m4_t[:], in_=m4[:])
                id_t = cpool.tile([128, 128], F16)
                nc.sync.dma_start(out=id_t[:], in_=ident[:])
                bqk_t = cpool.tile([1, E], F32R)
                nc.sync.dma_start(out=bqk_t[:], in_=bqk[:].bitcast(F32R))
                confr_t = cpool.tile([1, ppc], F32R)
                nc.sync.dma_start(out=confr_t[:], in_=confr[:].bitcast(F32R))
                sel_t = cpool.tile([128, 32, 128], F16)
                nc.sync.dma_start(out=sel_t[:], in_=sel[:])
                # zero-padded per-group weight columns; nonzero 4-col slices
                # are rewritten each sub-batch, the zero columns never change
                wd32_t = cpool.tile([128, 8, 32], F16)
                nc.sync.dma_start(out=wd32_t[:], in_=wd0[:])
                ones_t = cpool.tile([128, 8], F16)
                nc.sync.dma_start(out=ones_t[:], in_=ones1[:])

                c2t_t = c2pool.tile([128, 4, 2, ppc], F32R)
                nc.sync.dma_start(
                    out=c2t_t[:],
                    in_=cur2t[:].bitcast(F32R).rearrange(
                        "(ka kb p) n -> p ka kb n", p=128, kb=2
                    ),
                )

                wqk_t = wpool.tile([128, 8, E], F32R, tag="bigw")
                nc.sync.dma_start(
                    out=wqk_t[:],
                    in_=wqk[:].bitcast(F32R).rearrange("(k p) e -> p k e", p=128),
                )

                # ---- phase A: Qk = conf*scale*(cur2 @ Wqk + bqk), f16 ----
                qks = []
                for b in range(nb):
                    qk_t = qkpool.tile([128, E], F16, tag="qk")
                    for h in range(2):
                        ps = ps_sh.tile([128, 512], F32, tag="mm512")
                        for k in range(8):
                            nc.tensor.matmul(
                                ps[:],
                                lhsT=(
                                    c2t_t[:, k // 2, k % 2, 128 * b : 128 * (b + 1)]
                                ),
                                rhs=(wqk_t[:, k, 512 * h : 512 * (h + 1)]),
                                start=(k == 0),
                                stop=False,
                            )
                            # bias: out[p,:] += confr[p] * bqk  (conf*scale fold)
                        nc.tensor.matmul(
                            ps[:],
                            lhsT=(confr_t[:, 128 * b : 128 * (b + 1)]),
                            rhs=(bqk_t[:, 512 * h : 512 * (h + 1)]),
                            start=False,
                            stop=True,
                        )
                        nc.scalar.activation(
                            qk_t[:, 512 * h : 512 * (h + 1)], ps[:], AF.Copy
                        )
                    qks.append(qk_t)
                    if DEBUG_STOP == "A":
                        nc.sync.dma_start(
                            out=out[:]
                            .rearrange("(bb p) d two -> p bb (d two)", p=128)
                            [:, b, 0:512],
                            in_=qk_t[:].bitcast(F32),
                        )

                # vW reuses the Wqk SBUF slot once phase A has consumed it
                vw_t = wpool.tile([128, 8, E], F16, tag="bigw")
                nc.sync.dma_start(
                    out=vw_t[:],
                    in_=vw[:].rearrange("(k p) e -> p k e", p=128),
                )

                # ---- phases B (scores/softmax/hbar) + C (ctx/out) ----
                # Software-pipelined over 4*nb sub-batches: the score phase of
                # sub-batch i+1 is emitted BEFORE the hbar phase of i, so PE
                # has qkr(i+1) ready while DVE drains scores(i) and the
                # exp8->wd32->hb32 round trip of i never stalls the DVE queue.
                batch_st: dict[int, dict] = {}

                def emit_scores(b, sb):
                    if sb == 0:
                        cur_t = curpool.tile([128, 2, D], F32, tag="cur")
                        nc.sync.dma_start(
                            out=cur_t[:, 0, :],
                            in_=cur_r[128 * b : 128 * (b + 1), :],
                        )
                        nc.sync.dma_start(
                            out=cur_t[:, 1, :],
                            in_=cur_i[128 * b : 128 * (b + 1), :],
                        )
                        hbar_b = hbpool.tile([128, E], F16)
                        batch_st[b] = {"cur": cur_t, "hbar": hbar_b}
                    if ABLATE == "honce" and sb > 0:
                        h_t = batch_st[b]["h0"]
                    else:
                        h_t = hpool.tile([128, 8, E], F16, tag="h")
                        p0 = 128 * b + 32 * sb
                        nc.sync.dma_start(
                            out=h_t[:, :, 0:D],
                            in_=hist_r[p0 : p0 + 32].rearrange(
                                "(gl j) s d -> (j s) gl d", j=4
                            ),
                        )
                        nc.scalar.dma_start(
                            out=h_t[:, :, D:E],
                            in_=hist_i[p0 : p0 + 32].rearrange(
                                "(gl j) s d -> (j s) gl d", j=4
                            ),
                        )
                        if sb == 0:
                            batch_st[b]["h0"] = h_t

                    scores8 = smpool.tile([128, 8], F32, tag="scores")
                    exp8 = smpool.tile([128, 8], F32, tag="exp")
                    if ABLATE == "noscore":
                        nc.vector.memset(scores8[:], 0.5)
                    for gl in range(8 if ABLATE != "noscore" else 0):
                        qkr = ps_rep.tile([128, E], F32)
                        for h in range(2):
                            nc.tensor.matmul(
                                qkr[:, 512 * h : 512 * (h + 1)],
                                lhsT=(sel_t[:, 8 * sb + gl, :]),
                                rhs=(qks[b][:, 512 * h : 512 * (h + 1)]),
                                start=True,
                                stop=True,
                            )
                        # scores: elementwise H*Qk_rep then free-axis accum.
                        # Pool cannot read PSUM, so groups 0-2 get an ACT
                        # copy of qkr into SBUF f16 and run on Pool; groups
                        # 3-7 run on DVE straight from PSUM.
                        if gl < 3:
                            qkr_s = qkspool.tile([128, E], F16, tag="qkrs")
                            nc.scalar.activation(qkr_s[:], qkr[:], AF.Copy)
                            prod = prodpool.tile([128, E], F16, tag="prodp")
                            nc.gpsimd.tensor_tensor(
                                out=prod[:],
                                in0=h_t[:, gl, :],
                                in1=qkr_s[:],
                                op=AX.mult,
                            )
                            # free-axis reduce is DVE-only
                            sink = prodpool.tile(
                                [128, E], F16, tag="sinkp", bufs=1
                            )
                            nc.vector.tensor_scalar(
                                sink[:],
                                prod[:],
                                1.0,
                                None,
                                AX.mult,
                                op1=AX.add,
                                accum_out=scores8[:, gl : gl + 1],
                            )
                        else:
                            prod = prodpool.tile([128, E], F16, tag="prod")
                            nc.vector.tensor_tensor(
                                out=prod[:],
                                in0=h_t[:, gl, :],
                                in1=qkr[:],
                                op=AX.mult,
                            )
                            sink = prodpool.tile(
                                [128, E], F16, tag="sink", bufs=1
                            )
                            nc.vector.tensor_scalar(
                                sink[:],
                                prod[:],
                                1.0,
                                None,
                                AX.mult,
                                op1=AX.add,
                                accum_out=scores8[:, gl : gl + 1],
                            )
                    nc.scalar.activation(exp8[:], scores8[:], AF.Exp)
                    if DEBUG_STOP == "B":
                        nc.sync.dma_start(
                            out=out[:]
                            .rearrange("(bb p) d two -> p bb (d two)", p=128)
                            [:, b, 8 * sb : 8 * (sb + 1)],
                            in_=exp8[:],
                        )
                    return {"b": b, "sb": sb, "h": h_t, "exp8": exp8}

                def emit_hbar(st):
                    b, sb, h_t, exp8 = st["b"], st["sb"], st["h"], st["exp8"]
                    hbar_b = batch_st[b]["hbar"]
                    # hbar: accumulate all 8 groups into one [32, E] PSUM
                    # tile via zero-padded [128, 32] weight matmuls
                    for gl in range(8):
                        nc.vector.tensor_scalar_mul(
                            wd32_t[:, gl, 4 * gl : 4 * (gl + 1)],
                            m4_t[:],
                            exp8[:, gl : gl + 1],
                        )
                    hb32 = ps_hb.tile([32, E], F32)
                    for h in range(2):
                        for gl in range(8):
                            nc.tensor.matmul(
                                hb32[:, 512 * h : 512 * (h + 1)],
                                lhsT=wd32_t[:, gl, :],
                                rhs=h_t[:, gl, 512 * h : 512 * (h + 1)],
                                start=(gl == 0),
                                stop=(gl == 7),
                            )
                    # den32[q] = sum_p wd32_g[p, q] summed over groups
                    den32 = ps_sh.tile([32, 8], F32, tag="mm512")
                    for gl in range(8):
                        nc.tensor.matmul(
                            den32[:],
                            lhsT=wd32_t[:, gl, :],
                            rhs=ones_t[:],
                            start=(gl == 0),
                            stop=(gl == 7),
                        )
                    inv32 = dnpool.tile([32, 1], F32, tag="inv32")
                    nc.vector.reciprocal(inv32[:], den32[:, 0:1])
                    hsb32 = hsbpool.tile([32, E], F16)
                    nc.scalar.activation(
                        hsb32[:], hb32[:], AF.Copy, scale=inv32[:]
                    )
                    nc.gpsimd.dma_start(
                        out=hbar_b[32 * sb : 32 * (sb + 1), :], in_=hsb32[:]
                    )

                def emit_batch_end(b):
                    hbar_b = batch_st[b]["hbar"]
                    cur_t = batch_st[b]["cur"]
                    if DEBUG_STOP == "HB":
                        nc.sync.dma_start(
                            out=out[:]
                            .rearrange("(bb p) d two -> p bb (d two)", p=128)
                            [:, b, 512:1024],
                            in_=hbar_b[:].bitcast(F32),
                        )
                    # transpose hbar [128 pairs, E] -> hbarT [128 e, 128 p]
                    hts = []
                    for c in range(8):
                        tp = ps_sh.tile([128, 128], F16, tag="mm512")
                        nc.tensor.transpose(
                            tp[:], hbar_b[:, 128 * c : 128 * (c + 1)], id_t[:]
                        )
                        ht = htpool.tile([128, 128], F16, tag="hbarT")
                        nc.scalar.activation(ht[:], tp[:], AF.Copy)
                        hts.append(ht)

                    out_t = outpool.tile([128, D, 2], F32)
                    for h2 in range(2):
                        cps = ps_sh.tile([128, 512], F32, tag="mm512")
                        for c in range(8):
                            nc.tensor.matmul(
                                cps[:],
                                lhsT=(hts[c][:]),
                                rhs=(vw_t[:, c, 512 * h2 : 512 * (h2 + 1)]),
                                start=(c == 0),
                                stop=(c == 7),
                            )
                        nc.vector.scalar_tensor_tensor(
                            out=out_t[:, :, h2],
                            in0=cps[:],
                            scalar=0.1,
                            in1=cur_t[:, h2, :],
                            op0=AX.mult,
                            op1=AX.add,
                        )
                    if not DEBUG_STOP:
                        nc.sync.dma_start(
                            out=out[:]
                            .rearrange("(b p) d two -> p b d two", p=128)[:, b],
                            in_=out_t[:],
                        )

                pend = None
                for b in range(nb):
                    for sb in range(4):
                        st = emit_scores(b, sb)
                        if pend is not None:
                            emit_hbar(pend)
                            if pend["sb"] == 3:
                                emit_batch_end(pend["b"])
                        pend = st
                emit_hbar(pend)
                emit_batch_end(pend["b"])

    # bacc lowering: splits multi-wait instructions (walrus allows only one
    # sync wait per instruction), register allocation, DCE
    nc.compile()
    return nc


_CACHE: dict[int, bass.Bass] = {}


def get_nc(ppc: int) -> bass.Bass:
    if ppc not in _CACHE:
        _CACHE[ppc] = build(ppc)
    return _CACHE[ppc]


def make_const_inputs():
    # sel[k, q, p] = 1 iff qk row k == pair 4q + p//32 of the batch
    sel_h = np.zeros((128, 32, 128), np.float16)
    for q in range(32):
        for j in range(4):
            sel_h[4 * q + j, q, 32 * j : 32 * (j + 1)] = 1.0
    m4_h = np.zeros((128, 4), np.float32)
    for j in range(4):
        m4_h[32 * j : 32 * (j + 1), j] = 1.0
    id_h = np.eye(128, dtype=np.float16)
    return sel_h, m4_h, id_h


def host_prep(hist_real, hist_imag, current_real, current_imag, confidence,
              qW, qb, kW, kb, vW, vb, ppc):
    """Shared host-side folding + per-core input maps."""
    f = lambda x: np.ascontiguousarray(np.asarray(x, dtype=np.float32))
    hist_real, hist_imag = f(hist_real), f(hist_imag)
    current_real, current_imag = f(current_real), f(current_imag)
    confidence = f(confidence)
    qW, qb, kW, kb, vW, vb = f(qW), f(qb), f(kW), f(kb), f(vW), f(vb)

    n_cores = (B * T) // ppc
    wqk_h = np.ascontiguousarray(qW @ kW.T)          # [D2, E]
    bqk_h = (qb @ kW.T).reshape(1, E)                # [1, E]
    vw_h = vW.astype(np.float16)
    sel_h, m4_h, id_h = make_const_inputs()

    hr = hist_real.reshape(B * T, S, D)
    hi = hist_imag.reshape(B * T, S, D)
    cr = current_real.reshape(B * T, D)
    ci = current_imag.reshape(B * T, D)
    cf = confidence.reshape(B * T)

    hr16 = hr.astype(np.float16)
    hi16 = hi.astype(np.float16)

    in_maps = []
    for c in range(n_cores):
        sl = slice(c * ppc, (c + 1) * ppc)
        cfs = cf[sl] * SCALE                          # [ppc]
        cur2t_h = np.ascontiguousarray(
            (np.concatenate([cr[sl], ci[sl]], axis=1) * cfs[:, None]).T
        )  # [D2, ppc], conf*scale folded into the columns
        in_maps.append({
            "hist_real": hr16[sl],
            "hist_imag": hi16[sl],
            "cur_r": cr[sl],
            "cur_i": ci[sl],
            "cur2t": cur2t_h,
            "confr": np.ascontiguousarray(cfs.reshape(1, ppc)),
            "wqk": wqk_h,
            "bqk": bqk_h,
            "vw": vw_h,
            "sel": sel_h,
            "m4": m4_h,
            "ident": id_h,
            "wd0": np.zeros((128, 8, 32), np.float16),
            "ones1": np.ones((128, 8), np.float16),
        })
    return in_maps


def postprocess(out_full, vb):
    """Add the host-folded 0.1*vb term (ctx bias) to the gathered output."""
    vb = np.asarray(vb, dtype=np.float32)
    if np.any(vb):
        out_full = out_full.copy()
        out_full[:, :, 0] += 0.1 * vb[:D]
        out_full[:, :, 1] += 0.1 * vb[D:]
    return out_full


def kernel(hist_real, hist_imag, current_real, current_imag, confidence,
           qW, qb, kW, kb, vW, vb):
    ppc = PAIRS // N_CORES
    nc = get_nc(ppc)
    in_maps = host_prep(hist_real, hist_imag, current_real, current_imag,
                        confidence, qW, qb, kW, kb, vW, vb, ppc)
    res = run_bass_kernel_spmd(nc, in_maps, list(range(N_CORES))).results
    out = np.concatenate([res[c]["out"] for c in range(N_CORES)], axis=0)
    out = postprocess(out, vb)
    return out.view(np.complex64)[..., 0].reshape(B, T, D)
